# revision 4
# baseline (speedup 1.0000x reference)
"""Trainium2 Bass kernel for nn_AttentionEnhancedBiLSTM (8 NeuronCores, SPMD).

Math (from the reference):
    x  = inputs[:, -1, :]                        # [B=1024, E=1024]
    af = softmax((x Wq^T)(x Wk^T)^T / 32) (x Wv^T) Wo^T + bo     (fwd dir)
    h_f = sigmoid(o) * tanh(sigmoid(i) * tanh(g)),  gates = (af+x) W_ih^T + b
    backward: same with xr = x[:, ::-1] and its own weights; output keeps the
    CELL state c_b = sigmoid(i)*tanh(g).
    out = concat([h_f, c_b], -1)                 # [1024, 1024]

Sharding: batch-sharded 8 ways (128 rows/core). Attention mixes the batch, so
each core computes k^T and v for its own rows and the full k^T/v are formed
with one AllGather per direction; everything else is local. Weights are
replicated (cheaper than TP: activation exchanges through DRAM cost more HBM
than the weight replication saves).

Schedule: kv_f -> AG_f -> kv_b -> AG_b -> q_f -> attn_f -> q_b -> attn_b, so
both collectives fly under local compute. Matmul operands are float32r
(single-pass fp32, full PE rate at moving dim >= 256); activations feeding a
matmul as the stationary operand are transposed on the PE.
"""

import numpy as np

import concourse.bass as bass
import concourse.mybir as mybir
import concourse.tile as tile
from concourse import bacc
from concourse.bass_utils import run_bass_kernel_spmd
from concourse.masks import make_identity

N_CORES = 8
B, T, E, H = 1024, 128, 1024, 512
BS = B // N_CORES          # 128 batch rows per core
NE = E // 128              # 8 e-chunks
F32 = mybir.dt.float32
FMM = mybir.dt.float32r


class _Dir:
    """Per-direction build state."""

    def __init__(self, d, ext, compute_h):
        self.d = d
        self.ext = ext
        self.compute_h = compute_h
        self.G = 3 * H if compute_h else 2 * H


def _emit(tc, nc, sb, ps, dram, ident, ones, dirs, out_sb, with_attn_bias):

    def mm_full(st, w_ext, b_ext, name, dma_eng):
        """psum[128, E] = x_shard @ W^T (+ b)  (lhsT = xT chunks, rhs = w)."""
        acc = ps.tile([128, E], F32, name=f"ps_{name}", tag="mm")
        for ec in range(NE):
            wt = sb.tile([128, E], FMM, name=f"w_{name}_{ec}", tag="w")
            dma_eng.dma_start(wt[:], w_ext[ec * 128:(ec + 1) * 128, :])
            for n in range(E // 512):
                nc.tensor.matmul(
                    acc[:, n * 512:(n + 1) * 512],
                    st.xT[:, ec * 128:(ec + 1) * 128],
                    wt[:, n * 512:(n + 1) * 512],
                    start=(ec == 0), stop=(ec == NE - 1 and not with_attn_bias),
                )
        if with_attn_bias:
            bt = sb.tile([1, E], FMM, name=f"b_{name}", tag="bias")
            nc.sync.dma_start(bt[:], b_ext[:])
            for n in range(E // 512):
                nc.tensor.matmul(
                    acc[:, n * 512:(n + 1) * 512],
                    ones[0:1, :],
                    bt[0:1, n * 512:(n + 1) * 512],
                    start=False, stop=True,
                )
        return acc

    def transpose_1024(src_sb, dst_name, dst_tag):
        """[128, 1024] natural -> [128, 1024] transposed-chunks via PE."""
        out = sb.tile([128, E], FMM, name=dst_name, tag=dst_tag)
        for half in range(2):
            tp = ps.tile([128, 512], FMM, name=f"tp_{dst_name}_{half}", tag="tp")
            for i in range(4):
                j = half * 4 + i
                nc.tensor.transpose(
                    tp[:, i * 128:(i + 1) * 128],
                    src_sb[:, j * 128:(j + 1) * 128],
                    ident[:],
                )
            nc.vector.tensor_copy(out[:, half * 512:(half + 1) * 512], tp[:])
        return out

    # ---- phase A (both dirs): local k^T and v shard + AllGather ----------
    for st in dirs:
        d, ext = st.d, st.ext
        st.xT = sb.tile([128, E], FMM, name=f"xT_{d}", tag=f"xT_{d}")
        nc.sync.dma_start(st.xT[:], ext["xT"].rearrange("(n p) m -> p n m", p=128))

        # bounce layout: rows [0:128) = k^T chunks as [p, jc*128+b];
        #                rows [128:256) = v natural [b, j]
        st.bounce_in = dram.tile([2 * BS, E], FMM, name=f"bin_{d}")
        st.bounce_out = dram.tile([N_CORES, 2 * BS, E], FMM, name=f"bout_{d}",
                                  addr_space="Shared")

        k_ps = mm_full(st, ext["wk"], ext["bk"], f"k{d}", nc.sync)
        k_sb = sb.tile([128, E], FMM, name=f"k_{d}", tag="act")
        for n in range(2):
            nc.vector.tensor_copy(k_sb[:, n * 512:(n + 1) * 512],
                                  k_ps[:, n * 512:(n + 1) * 512])
        kT = transpose_1024(k_sb, f"kT_{d}", "act2")
        nc.scalar.dma_start(st.bounce_in[0:BS, :], kT[:])

        v_ps = mm_full(st, ext["wv"], ext["bv"], f"v{d}", nc.scalar)
        v_sb = sb.tile([128, E], FMM, name=f"v_{d}", tag="act")
        for n in range(2):
            nc.vector.tensor_copy(v_sb[:, n * 512:(n + 1) * 512],
                                  v_ps[:, n * 512:(n + 1) * 512])
        nc.scalar.dma_start(st.bounce_in[BS:2 * BS, :], v_sb[:])

        nc.gpsimd.collective_compute(
            "AllGather",
            mybir.AluOpType.bypass,
            replica_groups=[list(range(N_CORES))],
            ins=[st.bounce_in.opt()],
            outs=[st.bounce_out.opt()],
        )

    # ---- phases B + C per direction --------------------------------------
    for st in dirs:
        d, ext, G = st.d, st.ext, st.G

        # B: q^T (overlaps the collectives)
        q_ps = mm_full(st, ext["wq"], ext["bq"], f"q{d}", nc.sync)
        q_sb = sb.tile([128, E], FMM, name=f"q_{d}", tag="act")
        for n in range(2):
            nc.vector.tensor_copy(q_sb[:, n * 512:(n + 1) * 512],
                                  q_ps[:, n * 512:(n + 1) * 512])
        qT = transpose_1024(q_sb, f"qT_{d}", f"qT_{d}")

        # C: attention + LSTM cell
        # k^T full / v full, g-major free layout: [128, g*1024 + (jc*128+b | j)]
        kT_full = sb.tile([128, NE * E], FMM, name=f"kTf_{d}", tag="kT_full")
        v_full = sb.tile([128, NE * E], FMM, name=f"vf_{d}", tag="v_full")
        for g in range(N_CORES):
            nc.scalar.dma_start(kT_full[:, g * E:(g + 1) * E],
                                st.bounce_out[g, 0:BS, :])
            nc.sync.dma_start(v_full[:, g * E:(g + 1) * E],
                                st.bounce_out[g, BS:2 * BS, :])

        # scores[b, b'] += qT[jc]^T @ kT[jc, b']; b' = g*128 + b_local
        kT_g = kT_full.rearrange("p (g x) -> p g x", g=N_CORES)
        scores = ps.tile([128, B], F32, name=f"scores_{d}", tag="mm")
        for jc in range(NE):
            for n in range(B // 512):
                nc.tensor.matmul(
                    scores[:, n * 512:(n + 1) * 512],
                    qT[:, jc * 128:(jc + 1) * 128],
                    kT_g[:, 4 * n:4 * (n + 1), jc * 128:(jc + 1) * 128],
                    start=(jc == 0), stop=(jc == NE - 1),
                )

        # softmax along free axis (scores pre-scaled by 1/32 via wq)
        negmax = sb.tile([128, 1], F32, name=f"negmax_{d}", tag="stat")
        nc.vector.reduce_max(out=negmax[:], in_=scores[:],
                             axis=mybir.AxisListType.X, negate=True)
        p_sb = sb.tile([128, B], FMM, name=f"p_{d}", tag="act")
        rowsum = sb.tile([128, 1], F32, name=f"rowsum_{d}", tag="stat")
        nc.scalar.activation(p_sb[:], scores[:], mybir.ActivationFunctionType.Exp,
                             bias=negmax[:], scale=1.0, accum_out=rowsum[:])
        rinv = sb.tile([128, 1], F32, name=f"rinv_{d}", tag="stat")
        nc.vector.reciprocal(rinv[:], rowsum[:])

        pT = transpose_1024(p_sb, f"pT_{d}", "act2")
        av_ps = ps.tile([128, E], F32, name=f"av_{d}", tag="mm")
        for bc in range(NE):
            for n in range(E // 512):
                nc.tensor.matmul(
                    av_ps[:, n * 512:(n + 1) * 512],
                    pT[:, bc * 128:(bc + 1) * 128],
                    v_full[:, bc * E + n * 512: bc * E + (n + 1) * 512],
                    start=(bc == 0), stop=(bc == NE - 1),
                )
        av_sb = sb.tile([128, E], FMM, name=f"avn_{d}", tag="act")
        nc.vector.tensor_scalar_mul(av_sb[:], av_ps[:], rinv[:])

        avT = transpose_1024(av_sb, f"avT_{d}", "act2")
        af_ps = ps.tile([128, E], F32, name=f"af_{d}", tag="mm")
        for jc in range(NE):
            wot = sb.tile([128, E], FMM, name=f"wo_{d}_{jc}", tag="w")
            nc.sync.dma_start(wot[:], ext["wo"][jc * 128:(jc + 1) * 128, :])
            for n in range(E // 512):
                nc.tensor.matmul(
                    af_ps[:, n * 512:(n + 1) * 512],
                    avT[:, jc * 128:(jc + 1) * 128],
                    wot[:, n * 512:(n + 1) * 512],
                    start=(jc == 0), stop=(jc == NE - 1 and not with_attn_bias),
                )
        if with_attn_bias:
            bo_sb = sb.tile([1, E], FMM, name=f"bo_{d}", tag="bias")
            nc.sync.dma_start(bo_sb[:], ext["bo"][:])
            for n in range(E // 512):
                nc.tensor.matmul(
                    af_ps[:, n * 512:(n + 1) * 512],
                    ones[0:1, :],
                    bo_sb[0:1, n * 512:(n + 1) * 512],
                    start=False, stop=True,
                )

        # lstm_in = af + x (natural), then transpose for the gates matmul
        x_sb = sb.tile([128, E], F32, name=f"x_{d}", tag="act")
        nc.sync.dma_start(x_sb[:], ext["x"][:])
        lstm_sb = sb.tile([128, E], FMM, name=f"lstm_{d}", tag="act")
        nc.vector.tensor_add(lstm_sb[:], af_ps[:], x_sb[:])
        lstmT = transpose_1024(lstm_sb, f"lstmT_{d}", "act2")

        gates = ps.tile([128, G], F32, name=f"gates_{d}", tag="mm")
        for ec in range(NE):
            wih = sb.tile([128, G], FMM, name=f"wih_{d}_{ec}", tag="w")
            nc.scalar.dma_start(wih[:], ext["wih"][ec * 128:(ec + 1) * 128, :])
            for n in range(G // 512):
                nc.tensor.matmul(
                    gates[:, n * 512:(n + 1) * 512],
                    lstmT[:, ec * 128:(ec + 1) * 128],
                    wih[:, n * 512:(n + 1) * 512],
                    start=(ec == 0), stop=False,
                )
        bih = sb.tile([1, G], FMM, name=f"bih_{d}", tag="bias")
        nc.sync.dma_start(bih[:], ext["bih"][:])
        for n in range(G // 512):
            nc.tensor.matmul(
                gates[:, n * 512:(n + 1) * 512],
                ones[0:1, :],
                bih[0:1, n * 512:(n + 1) * 512],
                start=False, stop=True,
            )

        # gate nonlinearities; c = sig(i)*tanh(g); fwd also h = sig(o)*tanh(c)
        Sig = mybir.ActivationFunctionType.Sigmoid
        Tanh = mybir.ActivationFunctionType.Tanh
        si = sb.tile([128, H], F32, name=f"si_{d}", tag="gate")
        nc.scalar.activation(si[:], gates[:, 0:H], Sig)
        tg = sb.tile([128, H], F32, name=f"tg_{d}", tag="gate")
        nc.scalar.activation(tg[:], gates[:, H:2 * H], Tanh)
        if st.compute_h:
            cst = sb.tile([128, H], F32, name=f"c_{d}", tag="gate")
            nc.vector.tensor_mul(cst[:], si[:], tg[:])
            tc_ = sb.tile([128, H], F32, name=f"tc_{d}", tag="gate")
            nc.scalar.activation(tc_[:], cst[:], Tanh)
            so = sb.tile([128, H], F32, name=f"so_{d}", tag="gate")
            nc.scalar.activation(so[:], gates[:, 2 * H:3 * H], Sig)
            nc.vector.tensor_mul(out_sb[:, 0:H], so[:], tc_[:])
        else:
            nc.vector.tensor_mul(out_sb[:, H:2 * H], si[:], tg[:])


def build_nc(with_attn_bias=False):
    nc = bacc.Bacc("TRN2", target_bir_lowering=False, debug=False,
                   num_devices=N_CORES)

    def din(name, shape, dt=FMM):
        return nc.dram_tensor(name, shape, dt, kind="ExternalInput").ap()

    ext = {}
    for d in ("f", "b"):
        ext[d] = {
            "xT": din(f"xT_{d}", [E, BS]),
            "x": din(f"x_{d}", [BS, E], F32),
            "wq": din(f"wq_{d}", [E, E]),
            "wk": din(f"wk_{d}", [E, E]),
            "wv": din(f"wv_{d}", [E, E]),
            "wo": din(f"wo_{d}", [E, E]),
            "bq": din(f"bq_{d}", [1, E]),
            "bk": din(f"bk_{d}", [1, E]),
            "bv": din(f"bv_{d}", [1, E]),
            "bo": din(f"bo_{d}", [1, E]),
        }
    ext["f"]["wih"] = din("wih_f", [E, 3 * H])
    ext["f"]["bih"] = din("bih_f", [1, 3 * H])
    ext["b"]["wih"] = din("wih_b", [E, 2 * H])
    ext["b"]["bih"] = din("bih_b", [1, 2 * H])
    out_ext = nc.dram_tensor("out", [BS, 2 * H], F32, kind="ExternalOutput").ap()

    with tile.TileContext(nc) as tc:
        with (
            tc.tile_pool(name="sb", bufs=1) as sb_pool,
            tc.tile_pool(name="ps", bufs=1, space="PSUM") as ps_pool,
            tc.tile_pool(name="dram", bufs=1, space="DRAM") as dram_pool,
        ):
            class P:
                def __init__(self, pool, defaults):
                    self.pool, self.defaults = pool, defaults

                def tile(self, shape, dtype, name=None, tag=""):
                    bufs = self.defaults.get(tag, 1)
                    return self.pool.tile(shape, dtype, name=name, tag=tag,
                                          bufs=bufs)

            sb = P(sb_pool, {"w": 5, "act": 4, "act2": 3, "bias": 2,
                             "gate": 6, "stat": 4})
            ps = P(ps_pool, {"mm": 2, "tp": 2})

            class D:
                def tile(self, shape, dtype, name=None, addr_space="Local"):
                    return dram_pool.tile(shape, dtype, name=name,
                                          addr_space=addr_space)

            dram = D()

            ident_f = sb_pool.tile([128, 128], F32, name="ident_f", tag="ident_f")
            make_identity(nc, ident_f)
            ones_f = sb_pool.tile([1, 128], F32, name="ones_f", tag="ones_f")
            nc.gpsimd.memset(ones_f[:], 1.0)
            ident = sb_pool.tile([128, 128], FMM, name="ident", tag="ident")
            nc.vector.tensor_copy(ident[:], ident_f[:])
            ones = sb_pool.tile([1, 128], FMM, name="ones", tag="ones")
            nc.vector.tensor_copy(ones[:], ones_f[:])

            out_sb = sb_pool.tile([BS, 2 * H], F32, name="out_sb", tag="out")

            dirs = [_Dir("f", ext["f"], True), _Dir("b", ext["b"], False)]
            _emit(tc, nc, sb, ps, dram, ident, ones, dirs, out_sb,
                  with_attn_bias)

            nc.sync.dma_start(out_ext[:], out_sb[:])

    nc.compile()
    return nc


_NC_CACHE = {}


def _get_nc(with_attn_bias=False):
    if with_attn_bias not in _NC_CACHE:
        _NC_CACHE[with_attn_bias] = build_nc(with_attn_bias)
    return _NC_CACHE[with_attn_bias]


def _prep_host(inputs, Wqkv, bqkv, Wo, bo, W_ih, b_ih, b_hh, flip):
    """Per-direction host-side tensors (shared across cores except x shards)."""
    c = np.ascontiguousarray
    x = inputs
    if flip:
        x = x[:, ::-1]
    wq = c(Wqkv[0:E].T.astype(np.float32) / 32.0)   # fold 1/sqrt(E)
    wk = c(Wqkv[E:2 * E].T.astype(np.float32))
    wv = c(Wqkv[2 * E:3 * E].T.astype(np.float32))
    bq = c(bqkv[0:E].reshape(1, E) / 32.0)
    bk = c(bqkv[E:2 * E].reshape(1, E))
    bv = c(bqkv[2 * E:3 * E].reshape(1, E))
    wo_t = c(Wo.T)
    bo_r = c(bo.reshape(1, E))
    blstm = b_ih + b_hh
    if flip:    # backward: only i and g gates are used
        wih = c(np.concatenate([W_ih[0:H], W_ih[2 * H:3 * H]], axis=0).T)
        bih = c(np.concatenate([blstm[0:H], blstm[2 * H:3 * H]]).reshape(1, -1))
    else:       # forward: i, g, o
        wih = c(np.concatenate([W_ih[0:H], W_ih[2 * H:3 * H],
                                W_ih[3 * H:4 * H]], axis=0).T)
        bih = c(np.concatenate([blstm[0:H], blstm[2 * H:3 * H],
                                blstm[3 * H:4 * H]]).reshape(1, -1))
    return x, dict(wq=wq, wk=wk, wv=wv, wo=wo_t, bq=bq, bk=bk, bv=bv,
                   bo=bo_r, wih=wih, bih=bih)


def build_in_maps(inputs_dict):
    """Per-core input maps from the full input dict (for test harness reuse)."""
    return _prepare(**inputs_dict)[0]


def _prepare(inputs, Wqkv_f, bqkv_f, Wo_f, bo_f, W_ih_f, b_ih_f, b_hh_f,
             Wqkv_b, bqkv_b, Wo_b, bo_b, W_ih_b, b_ih_b, b_hh_b):
    inputs = np.asarray(inputs, dtype=np.float32)
    x_last = np.ascontiguousarray(inputs[:, -1, :])          # [B, E]

    xf, shared_f = _prep_host(x_last, np.asarray(Wqkv_f), np.asarray(bqkv_f),
                              np.asarray(Wo_f), np.asarray(bo_f),
                              np.asarray(W_ih_f), np.asarray(b_ih_f),
                              np.asarray(b_hh_f), flip=False)
    xb, shared_b = _prep_host(x_last, np.asarray(Wqkv_b), np.asarray(bqkv_b),
                              np.asarray(Wo_b), np.asarray(bo_b),
                              np.asarray(W_ih_b), np.asarray(b_ih_b),
                              np.asarray(b_hh_b), flip=True)

    with_attn_bias = bool(
        np.any(np.asarray(bqkv_f)) or np.any(np.asarray(bo_f))
        or np.any(np.asarray(bqkv_b)) or np.any(np.asarray(bo_b)))

    c = np.ascontiguousarray
    in_maps = []
    for ci in range(N_CORES):
        rows = slice(ci * BS, (ci + 1) * BS)
        m = {"xT_f": c(xf[rows].T), "x_f": c(xf[rows]),
             "xT_b": c(xb[rows].T), "x_b": c(xb[rows])}
        for d, shared in (("f", shared_f), ("b", shared_b)):
            for k, v in shared.items():
                m[f"{k}_{d}"] = v
        in_maps.append(m)

    return in_maps, with_attn_bias


def kernel(**inputs):
    in_maps, with_attn_bias = _prepare(**inputs)
    nc = _get_nc(with_attn_bias)
    res = run_bass_kernel_spmd(nc, in_maps, core_ids=list(range(N_CORES)))
    out = np.concatenate([res.results[ci]["out"] for ci in range(N_CORES)],
                         axis=0)
    return out.astype(np.float32)



# revision 6
# speedup vs baseline: 2.4066x; 2.4066x over previous
"""Trainium2 Bass kernel for nn_AttentionEnhancedBiLSTM (8 NeuronCores, SPMD).

Math (from the reference), with the attention weights folded on the host:
    x  = inputs[:, -1, :]                               # [B=1024, E=1024]
    scores = x (Wq^T Wk / 32) x^T + w[None, :]          # Ms = Wq^T Wk / 32
    a  = softmax(scores)
    af = a (x (Wo Wv)^T) + r[None, :]                   # N = Wo Wv
    h/c = lstm_cell((af + x) W_ih^T + b)                # only live gates kept
The backward direction's feature flip x[:, ::-1] is folded into the host
weights (Ms[::-1, ::-1], etc.), so both directions read the same x / x^T.
Attention biases reduce to the per-column score bias w = x Wk^T bq / 32 and
a constant row r = Wo bv + bo added to the residual (host-folded into x).

Sharding: batch-sharded 8 ways (128 rows/core). Attention mixes the batch,
so each core computes v' = x N^T for its rows and the full v' is formed with
one AllGather per direction; everything else is local. Folded weights are
replicated but half the size of the originals, and all matmul operands are
bf16, so per-core HBM traffic is ~21 MiB vs ~60 MiB for the naive version.
"""

import numpy as np
import ml_dtypes

import concourse.bass as bass
import concourse.mybir as mybir
import concourse.tile as tile
from concourse import bacc
from concourse.bass_utils import run_bass_kernel_spmd
from concourse.masks import make_identity

N_CORES = 8
B, T, E, H = 1024, 128, 1024, 512
BS = B // N_CORES          # 128 batch rows per core
NE = E // 128              # 8 e-chunks
F32 = mybir.dt.float32
BF16 = mybir.dt.bfloat16
BFNP = ml_dtypes.bfloat16


class _Dir:
    def __init__(self, d, ext, compute_h):
        self.d = d
        self.ext = ext
        self.compute_h = compute_h
        self.G = 3 * H if compute_h else 2 * H


def _emit(tc, nc, sb, ps, dram, ident, ones, xo, xTf, dirs, out_sb):

    def mm_stream(lhsT_chunk, w_ext, Gout, name, dma_eng, last_stop=True):
        """psum[128, Gout] = lhsT^T @ W, W streamed from HBM in row chunks."""
        acc = ps.tile([128, Gout], F32, name=f"ps_{name}", tag="mm")
        for ec in range(NE):
            wt = sb.tile([128, Gout], BF16, name=f"w_{name}_{ec}", tag="w")
            dma_eng.dma_start(wt[:], w_ext[ec * 128:(ec + 1) * 128, :])
            for n in range(Gout // 512):
                nc.tensor.matmul(
                    acc[:, n * 512:(n + 1) * 512],
                    lhsT_chunk(ec),
                    wt[:, n * 512:(n + 1) * 512],
                    start=(ec == 0),
                    stop=(ec == NE - 1 and last_stop),
                )
        return acc

    def add_bias_rows(acc, b_ext, Gout, name):
        """acc[128, Gout] += ones^T @ b (rank-1 broadcast of a bias row)."""
        bt = sb.tile([1, Gout], BF16, name=f"b_{name}", tag="bias")
        nc.sync.dma_start(bt[:], b_ext[:])
        for n in range(Gout // 512):
            nc.tensor.matmul(
                acc[:, n * 512:(n + 1) * 512],
                ones[0:1, :],
                bt[0:1, n * 512:(n + 1) * 512],
                start=False, stop=True,
            )

    def transpose_1024(src_sb, dst_name, dst_tag="act2"):
        """[128, 1024] natural bf16 -> [128, 1024] transposed-chunks bf16."""
        out = sb.tile([128, E], BF16, name=dst_name, tag=dst_tag)
        for half in range(2):
            tp = ps.tile([128, 512], BF16, name=f"tp_{dst_name}_{half}", tag="tp")
            for i in range(4):
                j = half * 4 + i
                nc.tensor.transpose(
                    tp[:, i * 128:(i + 1) * 128],
                    src_sb[:, j * 128:(j + 1) * 128],
                    ident[:],
                )
            nc.vector.tensor_copy(out[:, half * 512:(half + 1) * 512], tp[:])
        return out

    def psum_to_sb(acc, name, tag="act", dt=BF16):
        out = sb.tile([128, E], dt, name=name, tag=tag)
        for n in range(2):
            nc.vector.tensor_copy(out[:, n * 512:(n + 1) * 512],
                                  acc[:, n * 512:(n + 1) * 512])
        return out

    xo_chunk = lambda ec: xo[:, ec * BS:(ec + 1) * BS]

    # ---- phase A (both dirs): local v' shard + AllGather -----------------
    for st in dirs:
        d, ext = st.d, st.ext
        vp_ps = mm_stream(xo_chunk, ext["nv"], E, f"v{d}",
                          nc.scalar if d == "f" else nc.sync)
        v_own = psum_to_sb(vp_ps, f"vown_{d}", tag="vown")
        st.bounce_in = dram.tile([BS, E], BF16, name=f"bin_{d}")
        st.bounce_out = dram.tile([N_CORES, BS, E], BF16, name=f"bout_{d}",
                                  addr_space="Shared")
        nc.scalar.dma_start(st.bounce_in[:], v_own[:])
        nc.gpsimd.collective_compute(
            "AllGather",
            mybir.AluOpType.bypass,
            replica_groups=[list(range(N_CORES))],
            ins=[st.bounce_in.opt()],
            outs=[st.bounce_out.opt()],
        )

    # ---- phase B (both dirs): scores + softmax (overlaps the AllGathers) -
    for st in dirs:
        d, ext = st.d, st.ext
        s_ps = mm_stream(xo_chunk, ext["ms"], E, f"s{d}", nc.sync)
        s_sb = psum_to_sb(s_ps, f"s_{d}")
        sT = transpose_1024(s_sb, f"sT_{d}")

        scores = ps.tile([128, B], F32, name=f"scores_{d}", tag="mm")
        for ec in range(NE):
            for n in range(B // 512):
                nc.tensor.matmul(
                    scores[:, n * 512:(n + 1) * 512],
                    sT[:, ec * 128:(ec + 1) * 128],
                    xTf[:, ec * B + n * 512: ec * B + (n + 1) * 512],
                    start=(ec == 0), stop=False,
                )
        add_bias_rows(scores, ext["w"], B, f"w_{d}")

        negmax = sb.tile([128, 1], F32, name=f"negmax_{d}", tag="stat")
        nc.vector.reduce_max(out=negmax[:], in_=scores[:],
                             axis=mybir.AxisListType.X, negate=True)
        st.p_sb = sb.tile([128, B], BF16, name=f"p_{d}", tag="act")
        rowsum = sb.tile([128, 1], F32, name=f"rowsum_{d}", tag="stat")
        nc.scalar.activation(st.p_sb[:], scores[:],
                             mybir.ActivationFunctionType.Exp,
                             bias=negmax[:], scale=1.0, accum_out=rowsum[:])
        st.rinv = sb.tile([128, 1], F32, name=f"rinv_{d}", tag="stat")
        nc.vector.reciprocal(st.rinv[:], rowsum[:])
        st.pT = transpose_1024(st.p_sb, f"pT_{d}")

    # ---- phase C per direction: attention av + LSTM cell -----------------
    for st in dirs:
        d, ext, G = st.d, st.ext, st.G

        v_full = sb.tile([128, NE * E], BF16, name=f"vf_{d}", tag="v_full")
        for g in range(N_CORES):
            (nc.scalar if g % 2 else nc.sync).dma_start(
                v_full[:, g * E:(g + 1) * E], st.bounce_out[g, :, :])

        av_ps = ps.tile([128, E], F32, name=f"av_{d}", tag="mm")
        for bc in range(NE):
            for n in range(E // 512):
                nc.tensor.matmul(
                    av_ps[:, n * 512:(n + 1) * 512],
                    st.pT[:, bc * 128:(bc + 1) * 128],
                    v_full[:, bc * E + n * 512: bc * E + (n + 1) * 512],
                    start=(bc == 0), stop=(bc == NE - 1),
                )

        # lstm_in = av * rinv + x_eff  (bf16 for the gates matmul)
        xe_sb = sb.tile([128, E], BF16, name=f"xe_{d}", tag="xe")
        nc.sync.dma_start(xe_sb[:], ext["xe"][:])
        av_n = sb.tile([128, E], F32, name=f"avn_{d}", tag="avn")
        nc.vector.tensor_scalar_mul(av_n[:], av_ps[:], st.rinv[:])
        lstm_sb = sb.tile([128, E], BF16, name=f"lstm_{d}", tag="act")
        nc.vector.tensor_add(lstm_sb[:], av_n[:], xe_sb[:])
        lstmT = transpose_1024(lstm_sb, f"lstmT_{d}")

        gates = mm_stream(
            lambda ec: lstmT[:, ec * 128:(ec + 1) * 128],
            ext["wih"], G, f"g{d}", nc.scalar, last_stop=False)
        add_bias_rows(gates, ext["bih"], G, f"bih_{d}")

        Sig = mybir.ActivationFunctionType.Sigmoid
        Tanh = mybir.ActivationFunctionType.Tanh
        si = sb.tile([128, H], F32, name=f"si_{d}", tag="gate")
        nc.scalar.activation(si[:], gates[:, 0:H], Sig)
        tg = sb.tile([128, H], F32, name=f"tg_{d}", tag="gate")
        nc.scalar.activation(tg[:], gates[:, H:2 * H], Tanh)
        if st.compute_h:
            cst = sb.tile([128, H], F32, name=f"c_{d}", tag="gate")
            nc.vector.tensor_mul(cst[:], si[:], tg[:])
            tc_ = sb.tile([128, H], F32, name=f"tc_{d}", tag="gate")
            nc.scalar.activation(tc_[:], cst[:], Tanh)
            so = sb.tile([128, H], F32, name=f"so_{d}", tag="gate")
            nc.scalar.activation(so[:], gates[:, 2 * H:3 * H], Sig)
            nc.vector.tensor_mul(out_sb[:, 0:H], so[:], tc_[:])
        else:
            nc.vector.tensor_mul(out_sb[:, H:2 * H], si[:], tg[:])


def build_nc():
    nc = bacc.Bacc("TRN2", target_bir_lowering=False, debug=False,
                   num_devices=N_CORES)

    def din(name, shape, dt=BF16):
        return nc.dram_tensor(name, shape, dt, kind="ExternalInput").ap()

    ext = {}
    for d in ("f", "b"):
        G = 3 * H if d == "f" else 2 * H
        ext[d] = {
            "ms": din(f"ms_{d}", [E, E]),
            "nv": din(f"nv_{d}", [E, E]),
            "wih": din(f"wih_{d}", [E, G]),
            "bih": din(f"bih_{d}", [1, G]),
            "w": din(f"w_{d}", [1, B]),
            "xe": din(f"xe_{d}", [BS, E]),
        }
    xTo_ext = din("xTo", [E, BS])
    xTf_ext = din("xTf", [E, B])
    out_ext = nc.dram_tensor("out", [BS, 2 * H], F32, kind="ExternalOutput").ap()

    with tile.TileContext(nc) as tc:
        with (
            tc.tile_pool(name="sb", bufs=1) as sb_pool,
            tc.tile_pool(name="ps", bufs=1, space="PSUM") as ps_pool,
            tc.tile_pool(name="dram", bufs=1, space="DRAM") as dram_pool,
        ):
            class P:
                def __init__(self, pool, defaults):
                    self.pool, self.defaults = pool, defaults

                def tile(self, shape, dtype, name=None, tag=""):
                    bufs = self.defaults.get(tag, 1)
                    return self.pool.tile(shape, dtype, name=name, tag=tag,
                                          bufs=bufs)

            sb = P(sb_pool, {"w": 6, "act": 4, "act2": 4, "bias": 4,
                             "gate": 6, "stat": 6, "vown": 2, "avn": 2,
                             "xe": 2, "v_full": 2})
            ps = P(ps_pool, {"mm": 2, "tp": 2})

            class D:
                def tile(self, shape, dtype, name=None, addr_space="Local"):
                    return dram_pool.tile(shape, dtype, name=name,
                                          addr_space=addr_space)

            dram = D()

            ident_f = sb_pool.tile([128, 128], F32, name="ident_f",
                                   tag="ident_f")
            make_identity(nc, ident_f)
            ones_f = sb_pool.tile([1, 128], F32, name="ones_f", tag="ones_f")
            nc.gpsimd.memset(ones_f[:], 1.0)
            ident = sb_pool.tile([128, 128], BF16, name="ident", tag="ident")
            nc.vector.tensor_copy(ident[:], ident_f[:])
            ones = sb_pool.tile([1, 128], BF16, name="ones", tag="ones")
            nc.vector.tensor_copy(ones[:], ones_f[:])

            xo = sb_pool.tile([128, E], BF16, name="xo", tag="xo")
            nc.sync.dma_start(xo[:],
                              xTo_ext.rearrange("(n p) m -> p n m", p=128))
            xTf = sb_pool.tile([128, NE * B], BF16, name="xTf", tag="xTf")
            nc.scalar.dma_start(xTf[:],
                                xTf_ext.rearrange("(n p) m -> p n m", p=128))

            out_sb = sb_pool.tile([BS, 2 * H], F32, name="out_sb", tag="out")

            dirs = [_Dir("f", ext["f"], True), _Dir("b", ext["b"], False)]
            _emit(tc, nc, sb, ps, dram, ident, ones, xo, xTf, dirs, out_sb)

            nc.sync.dma_start(out_ext[:], out_sb[:])

    nc.compile()
    return nc


_NC_CACHE = {}


def _get_nc(variant=0):
    if variant not in _NC_CACHE:
        _NC_CACHE[variant] = build_nc()
    return _NC_CACHE[variant]


def _fold_dir(x, Wqkv, bqkv, Wo, bo, W_ih, b_ih, b_hh, flip):
    """Host-side weight folding for one direction. Returns f32 arrays."""
    c = np.ascontiguousarray
    Wq, Wk, Wv = Wqkv[0:E], Wqkv[E:2 * E], Wqkv[2 * E:3 * E]
    bq, bv = bqkv[0:E], bqkv[2 * E:3 * E]
    Ms = (Wq.T @ Wk) / 32.0                      # scores = x Ms x^T + w
    N = (Wo @ Wv).T                              # v' = x N  (rhs layout)
    r = Wo @ bv + bo                             # row bias folded into x
    gsel = (0, 2, 3) if not flip else (0, 2)     # live gates (i, g[, o])
    wih = np.concatenate([W_ih[g * H:(g + 1) * H] for g in gsel], 0).T
    blstm = b_ih + b_hh
    bih = np.concatenate([blstm[g * H:(g + 1) * H] for g in gsel])
    if flip:
        ms = Ms[::-1, ::-1]
        nv = N[::-1, ::-1]
        wih = wih[::-1, :]
        w = (x[:, ::-1] @ (Wk.T @ bq)) / 32.0
        xe = x + r[::-1][None, :]
    else:
        ms, nv = Ms, N
        w = x @ (Wk.T @ bq) / 32.0
        xe = x + r[None, :]
    return dict(ms=c(ms), nv=c(nv), wih=c(wih),
                bih=c(bih.reshape(1, -1)), w=c(w.reshape(1, B)), xe=xe)


def _prepare(inputs, Wqkv_f, bqkv_f, Wo_f, bo_f, W_ih_f, b_ih_f, b_hh_f,
             Wqkv_b, bqkv_b, Wo_b, bo_b, W_ih_b, b_ih_b, b_hh_b):
    f32 = lambda a: np.asarray(a, dtype=np.float32)
    x = np.ascontiguousarray(f32(inputs)[:, -1, :])          # [B, E]

    folds = {
        "f": _fold_dir(x, f32(Wqkv_f), f32(bqkv_f), f32(Wo_f), f32(bo_f),
                       f32(W_ih_f), f32(b_ih_f), f32(b_hh_f), flip=False),
        "b": _fold_dir(x, f32(Wqkv_b), f32(bqkv_b), f32(Wo_b), f32(bo_b),
                       f32(W_ih_b), f32(b_ih_b), f32(b_hh_b), flip=True),
    }
    bf = lambda a: np.ascontiguousarray(a.astype(BFNP))
    shared = {}
    for d, fo in folds.items():
        for k in ("ms", "nv", "wih", "bih", "w"):
            shared[f"{k}_{d}"] = bf(fo[k])
    xT = bf(x.T)

    in_maps = []
    for ci in range(N_CORES):
        rows = slice(ci * BS, (ci + 1) * BS)
        m = dict(shared)
        m["xTo"] = bf(np.ascontiguousarray(x[rows].T.astype(np.float32)))
        m["xTf"] = xT
        m["xe_f"] = bf(folds["f"]["xe"][rows])
        m["xe_b"] = bf(folds["b"]["xe"][rows])
        in_maps.append(m)
    return in_maps


def build_in_maps(inputs_dict):
    """Per-core input maps from the full input dict (for test harness reuse)."""
    return _prepare(**inputs_dict)


def kernel(**inputs):
    in_maps = _prepare(**inputs)
    nc = _get_nc()
    res = run_bass_kernel_spmd(nc, in_maps, core_ids=list(range(N_CORES)))
    out = np.concatenate([res.results[ci]["out"] for ci in range(N_CORES)],
                         axis=0)
    return out.astype(np.float32)
